# revision 13
# baseline (speedup 1.0000x reference)
"""Trainium2 Bass kernel for DBHDSNet multi-task detection loss.

Strategy (pure data parallel, B=16 over 8 cores, 2 samples/core):
 - Host (gt-only prep): target assignment replicated with jnp (bit-exact with
   the reference's build_targets on this backend), producing per-core padded
   positive-cell tables + gather row indices; zero-FLOP relayout of reg/cls
   predictions to (cell-major, channel) so positive rows are contiguous.
 - Device: all prediction-dependent math.
   * seg loss (the memory-bound bulk, 13.1MB/core): streaming chunks,
     softplus via Exp/Ln(x+1), sigma via Exp(-L); fused DVE STT reductions
     with accum_out; per-sample sum(t) via PE ones-matmul into PSUM.
   * obj dense softplus sums; positive-cell obj/reg/cls gathered with
     indirect row DMAs; CIoU + focal batched across the 3 scales.
   * hazard CE + expected-cost on [2,4].
 - Each core returns a [128,48] partial-sum tile; host does the final scalar
   reduction (the "all-reduce" of scalar losses).
"""
import sys
sys.path.insert(0, "/opt/trn_rl_repo")
import numpy as np

NUM_CLASSES = 38
GAMMA, ALPHA = 1.5, 0.25
EPS = 1e-7
L_BOX, L_OBJ, L_CLS, L_SEG, L_HAZ, L_HIER = 5.0, 1.0, 1.0, 2.0, 1.0, 0.5
SCALE_RANGES = [(0.0, 0.15), (0.1, 0.35), (0.25, 1.0)]
PENALTY = np.array([[0., 1., 2., 4.], [2., 0., 1., 2.],
                    [4., 2., 0., 1.], [8., 4., 2., 0.]], dtype=np.float32)
B = 16
NCORES = 8
BL = B // NCORES
HWS = [6400, 1600, 400]
DIMS = [(80, 80), (40, 40), (20, 20)]
K = 128
SEG_F = 6400
CHUNK = 1600
NCH = SEG_F // CHUNK
SEG_ELEMS = 32 * 160 * 160
NCOL = 48
CN = NUM_CLASSES

_CACHE = {}


def _build_program():
    import os
    SEC = os.environ.get("KSEC", "all")
    ABL = set(os.environ.get("KABL", "act,dve,pe").split(","))
    import concourse.bacc as bacc
    import concourse.bass as bass
    import concourse.mybir as mybir
    import concourse.tile as tile
    dt = mybir.dt
    AF = mybir.ActivationFunctionType
    OP = mybir.AluOpType

    # Route every Exp/Ln to the one table that holds both, so the act-table
    # pass emits a single load instead of one per Exp<->Ln transition.
    from concourse.hw_specs import get_activation_tables as _gat

    def _patched_tables(arch):
        tabs = _gat(arch)
        for name, s in tabs.items():
            if name != "natural_log_exp_and_others":
                s.discard(AF.Exp)
                s.discard(AF.Ln)
        return tabs
    bacc.get_activation_tables = _patched_tables

    nc = bacc.Bacc("TRN2", target_bir_lowering=False, debug=False,
                   num_devices=NCORES)

    def din(name, shape, dty=dt.float32):
        return nc.dram_tensor(name, shape, dty, kind="ExternalInput").ap()

    seg_x = din("seg_x", [BL, 128, SEG_F])
    seg_t = din("seg_t", [BL, 128, SEG_F])
    objd = [din("objd3", [128, 100]), din("objd4", [128, 25]),
            din("objd5", [100, 8])]
    clst = [din(f"clst{i}", [BL * HWS[i], CN]) for i in range(3)]
    regt = [din(f"regt{i}", [BL * HWS[i], 4]) for i in range(3)]
    objf = [din(f"objf{i}", [BL * HWS[i], 1]) for i in range(3)]
    idxs = [din(f"idx{i}", [K, 1], dt.int32) for i in range(3)]
    tboxall = din("tboxall", [K, 12])      # col = ch*3 + scale
    tclsall = din("tclsall", [K, 3 * CN])  # scale-contiguous
    atmall = din("atmall", [K, 3 * CN])
    bmaskall = din("bmaskall", [K, 3])
    haz_x = din("haz_x", [BL, 4])
    haz_oh = din("haz_oh", [BL, 4])
    haz_pr = din("haz_pr", [BL, 4])

    out = nc.dram_tensor("partials", [128, NCOL], dt.float32,
                         kind="ExternalOutput").ap()

    with tile.TileContext(nc) as tc:
        with tc.tile_pool(name="acc", bufs=1) as accp, \
             tc.tile_pool(name="xin", bufs=4) as xp, \
             tc.tile_pool(name="tin", bufs=4) as tp, \
             tc.tile_pool(name="work", bufs=3) as wp, \
             tc.tile_pool(name="junk", bufs=2) as jp, \
             tc.tile_pool(name="small", bufs=1) as sp, \
             tc.tile_pool(name="psum", bufs=2, space="PSUM") as pp:

            acc = accp.tile([128, NCOL], dt.float32)
            nc.vector.memset(acc[:], 0.0)
            ones = sp.tile([128, 1], dt.float32)
            nc.vector.memset(ones[:], 1.0)

            # per-quantity accumulators (avoid cross-engine false deps)
            a_sp = sp.tile([128, 8], dt.float32)    # ACT: sum softplus
            nc.vector.memset(a_sp[:], 0.0)
            a_r = sp.tile([128, 8], dt.float32)     # ACT: sum (1-sigma)
            nc.vector.memset(a_r[:], 0.0)
            a_xt = sp.tile([128, 8], dt.float32)    # DVE: sum x*t
            nc.vector.memset(a_xt[:], 0.0)
            a_rt = sp.tile([128, 8], dt.float32)    # DVE: sum (1-sigma)*t
            nc.vector.memset(a_rt[:], 0.0)
            a_ts = sp.tile([1, 2], dt.float32)      # DVE: sum t per sample
            a_ob = sp.tile([128, 3], dt.float32)    # ACT: obj softplus sums
            nc.vector.memset(a_ob[:], 0.0)
            a_ms = sp.tile([128, 8], dt.float32)    # DVE: objpos 0-2, ciou 3-5
            a_fc = sp.tile([128, 3], dt.float32)    # DVE: focal sums
            a_hz = sp.tile([BL, 2], dt.float32)     # DVE: hazard ce, hier

            # ---------------- seg main loop ----------------
            pst = [pp.tile([1, 512], dt.float32, name=f"pst{s}",
                           tag=f"pst{s}") for s in range(BL)]
            pending = None  # (r_tile, t_tile, col) for pipelined j2
            for s in range(BL):
                for c in range(NCH):
                    xt = xp.tile([128, CHUNK], dt.float32, tag="x")
                    nc.sync.dma_start(xt[:], seg_x[s, :, c*CHUNK:(c+1)*CHUNK])
                    tt = tp.tile([128, CHUNK], dt.float32, tag="t")
                    nc.sync.dma_start(tt[:], seg_t[s, :, c*CHUNK:(c+1)*CHUNK])
                    col = s * NCH + c
                    e = wp.tile([128, CHUNK], dt.float32, tag="e")
                    if "act" not in ABL:
                        continue
                    nc.scalar.activation(e[:], xt[:], AF.Exp)
                    L = wp.tile([128, CHUNK], dt.float32, tag="L")
                    nc.scalar.activation(L[:], e[:], AF.Ln, bias=1.0,
                                         accum_out=a_sp[:, col:col+1])
                    r = wp.tile([128, CHUNK], dt.float32, tag="r")
                    nc.scalar.activation(r[:], L[:], AF.Exp, scale=-1.0,
                                         accum_out=a_r[:, col:col+1])
                    j1 = jp.tile([128, CHUNK], dt.float32, tag="j1")
                    if "dve" in ABL:
                        nc.vector.scalar_tensor_tensor(
                            out=j1[:], in0=xt[:], scalar=1.0, in1=tt[:],
                            op0=OP.mult, op1=OP.mult,
                            accum_out=a_xt[:, col:col+1])
                    # j2 for the PREVIOUS chunk: its r is ready, so the DVE
                    # stream never stalls behind this chunk's 3-deep ACT chain
                    if pending is not None and "dve" in ABL:
                        pr, ptt, pcol = pending
                        j2 = jp.tile([128, CHUNK], dt.float32, tag="j2")
                        nc.vector.scalar_tensor_tensor(
                            out=j2[:], in0=pr[:], scalar=1.0, in1=ptt[:],
                            op0=OP.mult, op1=OP.mult,
                            accum_out=a_rt[:, pcol:pcol+1])
                    pending = (r, tt, col)
                    if "pe" in ABL:
                        nfull = CHUNK // 512
                        for j in range(nfull):
                            nc.tensor.matmul(pst[s][:, :512], lhsT=ones[:],
                                             rhs=tt[:, j*512:(j+1)*512],
                                             start=(c == 0 and j == 0),
                                             stop=False)
                        rem = CHUNK - nfull * 512
                        nc.tensor.matmul(pst[s][:, :rem], lhsT=ones[:],
                                         rhs=tt[:, nfull*512:CHUNK],
                                         start=False, stop=(c == NCH - 1))
            if "dve" in ABL:
                pr, ptt, pcol = pending
                j2 = jp.tile([128, CHUNK], dt.float32, tag="j2")
                nc.vector.scalar_tensor_tensor(
                    out=j2[:], in0=pr[:], scalar=1.0, in1=ptt[:],
                    op0=OP.mult, op1=OP.mult,
                    accum_out=a_rt[:, pcol:pcol+1])
            if "pe" in ABL:
                for s in range(BL):
                    nc.vector.tensor_reduce(a_ts[0:1, s:s+1], pst[s][:],
                                            axis=mybir.AxisListType.X,
                                            op=OP.add)

            # ---------------- obj dense softplus ----------------
            for i in range(3 if SEC in ("all", "obj", "sparse") else 0):
                p, f = objd[i].shape
                ot = sp.tile([p, f], dt.float32, tag=f"objd{i}")
                nc.sync.dma_start(ot[:], objd[i][:])
                oe = sp.tile([p, f], dt.float32, tag=f"obje{i}")
                nc.scalar.activation(oe[:], ot[:], AF.Exp)
                ol = sp.tile([p, f], dt.float32, tag=f"objl{i}")
                nc.scalar.activation(ol[:], oe[:], AF.Ln, bias=1.0,
                                     accum_out=a_ob[0:p, i:i+1])

            # ---------------- gathers + batched sparse ----------------
            if SEC in ("all", "sparse"):
                tba = sp.tile([K, 12], dt.float32)
                nc.sync.dma_start(tba[:], tboxall[:])
                tca = sp.tile([K, 3 * CN], dt.float32)
                nc.sync.dma_start(tca[:], tclsall[:])
                ama = sp.tile([K, 3 * CN], dt.float32)
                nc.sync.dma_start(ama[:], atmall[:])
                bma = sp.tile([K, 3], dt.float32)
                nc.sync.dma_start(bma[:], bmaskall[:])

                gca = sp.tile([K, 3 * CN], dt.float32)
                goa = sp.tile([K, 3], dt.float32)
                d4 = sp.tile([K, 12], dt.float32)
                for i in range(3):
                    it = sp.tile([K, 1], dt.int32, tag=f"idx{i}")
                    nc.sync.dma_start(it[:], idxs[i][:])
                    gr = sp.tile([K, 4], dt.float32, tag=f"gr{i}")
                    nc.gpsimd.indirect_dma_start(
                        out=gr[:], out_offset=None, in_=regt[i][:],
                        in_offset=bass.IndirectOffsetOnAxis(ap=it[:], axis=0))
                    nc.gpsimd.indirect_dma_start(
                        out=gca[:, i*CN:(i+1)*CN], out_offset=None,
                        in_=clst[i][:],
                        in_offset=bass.IndirectOffsetOnAxis(ap=it[:], axis=0))
                    nc.gpsimd.indirect_dma_start(
                        out=goa[:, i:i+1], out_offset=None, in_=objf[i][:],
                        in_offset=bass.IndirectOffsetOnAxis(ap=it[:], axis=0))
                    # sigmoid decode step 1: exp(-x), scale-interleaved cols
                    nc.scalar.activation(d4[:, i:12:3], gr[:], AF.Exp,
                                         scale=-1.0)

                # obj positive sums (all scales at once)
                nc.vector.tensor_tensor(out=a_ms[:, 0:3], in0=goa[:],
                                        in1=bma[:], op=OP.mult)

                # ---- decode finish: dec = 1/(1+exp(-x)), [K,12] ----
                nc.vector.tensor_scalar_add(d4[:], d4[:], 1.0)
                nc.vector.reciprocal(d4[:], d4[:])

                # ---- CIoU batched over 3 scales: ops on [K,3] ----
                w = sp.tile([K, 72], dt.float32)

                def col(n):
                    return w[:, 3*n:3*n+3]
                px, py, pw, ph = (d4[:, 3*m:3*m+3] for m in range(4))
                tx, ty, tw, th = (tba[:, 3*m:3*m+3] for m in range(4))
                phw, phh, thw, thh = col(0), col(1), col(2), col(3)
                nc.vector.tensor_scalar_mul(phw, pw, 0.5)
                nc.vector.tensor_scalar_mul(phh, ph, 0.5)
                nc.vector.tensor_scalar_mul(thw, tw, 0.5)
                nc.vector.tensor_scalar_mul(thh, th, 0.5)
                px1, px2, py1, py2 = col(4), col(5), col(6), col(7)
                nc.vector.tensor_sub(px1, px, phw)
                nc.vector.tensor_add(px2, px, phw)
                nc.vector.tensor_sub(py1, py, phh)
                nc.vector.tensor_add(py2, py, phh)
                tx1, tx2, ty1, ty2 = col(8), col(9), col(10), col(11)
                nc.vector.tensor_sub(tx1, tx, thw)
                nc.vector.tensor_add(tx2, tx, thw)
                nc.vector.tensor_sub(ty1, ty, thh)
                nc.vector.tensor_add(ty2, ty, thh)
                iw, ih = col(12), col(13)
                mn, mx = col(14), col(15)
                nc.vector.tensor_tensor(mn, px2, tx2, op=OP.min)
                nc.vector.tensor_tensor(mx, px1, tx1, op=OP.max)
                nc.vector.tensor_sub(iw, mn, mx)
                nc.vector.tensor_scalar_max(iw, iw, 0.0)
                nc.vector.tensor_tensor(mn, py2, ty2, op=OP.min)
                nc.vector.tensor_tensor(mx, py1, ty1, op=OP.max)
                nc.vector.tensor_sub(ih, mn, mx)
                nc.vector.tensor_scalar_max(ih, ih, 0.0)
                inter = col(16)
                nc.vector.tensor_mul(inter, iw, ih)
                uni, aa1 = col(17), col(18)
                nc.vector.tensor_mul(uni, pw, ph)
                nc.vector.tensor_mul(aa1, tw, th)
                nc.vector.tensor_add(uni, uni, aa1)
                nc.vector.tensor_sub(uni, uni, inter)
                nc.vector.tensor_scalar_add(uni, uni, EPS)
                iou = col(19)
                nc.vector.reciprocal(iou, uni)
                nc.vector.tensor_mul(iou, iou, inter)
                cw2, ch2 = col(20), col(21)
                nc.vector.tensor_tensor(mn, px2, tx2, op=OP.max)
                nc.vector.tensor_tensor(mx, px1, tx1, op=OP.min)
                nc.vector.tensor_sub(cw2, mn, mx)
                nc.vector.tensor_mul(cw2, cw2, cw2)
                nc.vector.tensor_tensor(mn, py2, ty2, op=OP.max)
                nc.vector.tensor_tensor(mx, py1, ty1, op=OP.min)
                nc.vector.tensor_sub(ch2, mn, mx)
                nc.vector.tensor_mul(ch2, ch2, ch2)
                nc.vector.tensor_add(cw2, cw2, ch2)
                nc.vector.tensor_scalar_add(cw2, cw2, EPS)  # c2
                rho2 = col(22)
                nc.vector.tensor_sub(mn, px, tx)
                nc.vector.tensor_mul(mn, mn, mn)
                nc.vector.tensor_sub(mx, py, ty)
                nc.vector.tensor_mul(mx, mx, mx)
                nc.vector.tensor_add(rho2, mn, mx)
                rc2 = col(23)
                nc.vector.reciprocal(rc2, cw2)
                nc.vector.tensor_mul(rho2, rho2, rc2)  # rho2/c2
                at = sp.tile([K, 24], dt.float32)
                q1, q2 = at[:, 0:3], at[:, 3:6]
                nc.vector.tensor_scalar_add(q1, th, EPS)
                nc.vector.reciprocal(q1, q1)
                nc.vector.tensor_mul(q1, q1, tw)
                nc.vector.tensor_scalar_add(q2, ph, EPS)
                nc.vector.reciprocal(q2, q2)
                nc.vector.tensor_mul(q2, q2, pw)

                # ---- focal batched on [K, 114] ----
                fe = sp.tile([K, 3 * CN], dt.float32)
                nc.scalar.activation(fe[:], gca[:], AF.Exp)
                fL = sp.tile([K, 3 * CN], dt.float32)
                nc.scalar.activation(fL[:], fe[:], AF.Ln, bias=1.0)
                fr = sp.tile([K, 3 * CN], dt.float32)
                nc.scalar.activation(fr[:], fL[:], AF.Exp, scale=-1.0)
                u = sp.tile([K, 3 * CN], dt.float32)
                nc.vector.tensor_scalar(out=u[:], in0=tca[:], scalar1=-2.0,
                                        scalar2=1.0, op0=OP.mult, op1=OP.add)
                q = sp.tile([K, 3 * CN], dt.float32)
                nc.vector.tensor_mul(q[:], fr[:], u[:])
                nc.vector.tensor_sub(q[:], u[:], q[:])
                nc.vector.tensor_add(q[:], q[:], tca[:])
                lq = sp.tile([K, 3 * CN], dt.float32)
                nc.scalar.activation(lq[:], q[:], AF.Ln)
                fw = sp.tile([K, 3 * CN], dt.float32)
                nc.scalar.activation(fw[:], lq[:], AF.Exp, scale=GAMMA)
                ce = sp.tile([K, 3 * CN], dt.float32)
                nc.vector.tensor_mul(ce[:], gca[:], tca[:])
                nc.vector.tensor_sub(ce[:], fL[:], ce[:])
                nc.vector.tensor_mul(fw[:], fw[:], ce[:])
                fj = sp.tile([K, CN], dt.float32)
                for i in range(3):
                    nc.vector.scalar_tensor_tensor(
                        out=fj[:], in0=fw[:, i*CN:(i+1)*CN], scalar=1.0,
                        in1=ama[:, i*CN:(i+1)*CN],
                        op0=OP.mult, op1=OP.mult,
                        accum_out=a_fc[:, i:i+1])

                # ---------------- hazard ----------------
                hx = sp.tile([BL, 4], dt.float32, tag="hx")
                nc.sync.dma_start(hx[:], haz_x[:])
                hoh = sp.tile([BL, 4], dt.float32, tag="hoh")
                nc.sync.dma_start(hoh[:], haz_oh[:])
                hpr = sp.tile([BL, 4], dt.float32, tag="hpr")
                nc.sync.dma_start(hpr[:], haz_pr[:])
                hm = sp.tile([BL, 8], dt.float32, tag="hm")
                mxc, nmx, Z, lz, s1, ce_, ec, hj = (
                    hm[:, m:m+1] for m in range(8))
                nc.vector.tensor_reduce(mxc, hx[:],
                                        axis=mybir.AxisListType.X, op=OP.max)
                nc.vector.tensor_scalar_mul(nmx, mxc, -1.0)
                he = sp.tile([BL, 4], dt.float32, tag="he")
                nc.scalar.activation(he[:], hx[:], AF.Exp, bias=nmx,
                                     accum_out=Z)
                nc.scalar.activation(lz, Z, AF.Ln)
                nc.vector.tensor_add(lz, lz, mxc)
                hj2 = sp.tile([BL, 4], dt.float32, tag="hj2")
                nc.vector.scalar_tensor_tensor(
                    out=hj2[:], in0=hx[:], scalar=1.0, in1=hoh[:],
                    op0=OP.mult, op1=OP.mult, accum_out=s1)
                nc.vector.tensor_sub(a_hz[:, 0:1], lz, s1)
                hj3 = sp.tile([BL, 4], dt.float32, tag="hj3")
                nc.vector.scalar_tensor_tensor(
                    out=hj3[:], in0=he[:], scalar=1.0, in1=hpr[:],
                    op0=OP.mult, op1=OP.mult, accum_out=ec)
                nc.vector.reciprocal(hj, Z)
                nc.vector.tensor_mul(ec, ec, hj)
                nc.vector.tensor_mul(a_hz[:, 1:2], ec, a_hz[:, 0:1])

                # ---- arctan tail (trig act table) + ciou finish ----
                a1v, a2v = at[:, 6:9], at[:, 9:12]
                nc.scalar.activation(a1v, q1, AF.Arctan)
                nc.scalar.activation(a2v, q2, AF.Arctan)
                dv, v = at[:, 12:15], at[:, 15:18]
                nc.vector.tensor_sub(dv, a1v, a2v)
                nc.vector.tensor_mul(v, dv, dv)
                nc.vector.tensor_scalar_mul(v, v, 4.0 / np.pi ** 2)
                den, al = at[:, 18:21], at[:, 21:24]
                nc.vector.tensor_scalar(out=den[:], in0=iou, scalar1=-1.0,
                                        scalar2=1.0 + EPS, op0=OP.mult,
                                        op1=OP.add)
                nc.vector.tensor_add(den, den, v)
                nc.vector.reciprocal(den, den)
                nc.vector.tensor_mul(al, v, den)
                nc.vector.tensor_mul(al, al, v)
                cio = at[:, 0:3]
                nc.vector.tensor_scalar(out=cio[:], in0=iou, scalar1=-1.0,
                                        scalar2=1.0, op0=OP.mult, op1=OP.add)
                nc.vector.tensor_add(cio, cio, rho2)
                nc.vector.tensor_add(cio, cio, al)
                nc.vector.tensor_tensor(out=a_ms[:, 3:6], in0=cio,
                                        in1=bma[:], op=OP.mult)

            # ---------------- assemble acc and store ----------------
            nc.vector.tensor_copy(acc[:, 0:8], a_sp[:])
            nc.vector.tensor_copy(acc[:, 8:16], a_r[:])
            nc.vector.tensor_copy(acc[:, 16:24], a_xt[:])
            nc.vector.tensor_copy(acc[:, 24:32], a_rt[:])
            nc.vector.tensor_copy(acc[0:1, 32:34], a_ts[:])
            nc.vector.tensor_copy(acc[:, 34:37], a_ob[:])
            if SEC in ("all", "sparse"):
                nc.vector.tensor_copy(acc[:, 37:40], a_ms[:, 0:3])
                nc.vector.tensor_copy(acc[:, 40:43], a_ms[:, 3:6])
                nc.vector.tensor_copy(acc[:, 43:46], a_fc[:])
                nc.vector.tensor_copy(acc[0:BL, 46:48], a_hz[:])
            nc.sync.dma_start(out[:], acc[:])

    nc.compile()
    return nc


def _jnp_targets(gt_boxes, gt_cls):
    """Replicate reference build_targets with jnp (oracle-exact semantics)."""
    import jax.numpy as jnp
    res = []
    for (H, W), (lo, hi) in zip(DIMS, SCALE_RANGES):
        gb = jnp.asarray(gt_boxes)
        gc = jnp.asarray(gt_cls)
        cx, cy, w, h = gb[..., 0], gb[..., 1], gb[..., 2], gb[..., 3]
        max_side = jnp.maximum(w, h)
        valid = (max_side >= lo) & (max_side < hi)
        gx = jnp.clip((cx * W).astype(jnp.int32), 0, W - 1)
        gy = jnp.clip((cy * H).astype(jnp.int32), 0, H - 1)
        flat = (jnp.arange(B, dtype=jnp.int32)[:, None] * (H * W)
                + gy * W + gx)
        flat = jnp.where(valid, flat, B * H * W).reshape(-1)
        tgt_reg = jnp.zeros((B * H * W + 1, 4), gb.dtype).at[flat].set(
            gb.reshape(-1, 4))[:B * H * W].reshape(B, H, W, 4)
        tgt_obj = jnp.zeros((B * H * W + 1,), gb.dtype).at[flat].set(
            1.0)[:B * H * W].reshape(B, H, W)
        tgt_cls = jnp.zeros((B * H * W + 1, NUM_CLASSES), gb.dtype).at[
            flat, gc.reshape(-1)].set(1.0)[:B * H * W].reshape(
            B, H, W, NUM_CLASSES)
        res.append((np.asarray(tgt_reg), np.asarray(tgt_obj),
                    np.asarray(tgt_cls)))
    return res


def kernel(**inputs):
    from concourse import bass_utils

    if "nc" not in _CACHE:
        _CACHE["nc"] = _build_program()
    nc = _CACHE["nc"]

    f32 = np.float32
    regs = [np.ascontiguousarray(inputs[f"reg_p{i+3}"], dtype=f32)
            for i in range(3)]
    objs = [np.ascontiguousarray(inputs[f"obj_p{i+3}"], dtype=f32)
            for i in range(3)]
    clss = [np.ascontiguousarray(inputs[f"cls_p{i+3}"], dtype=f32)
            for i in range(3)]
    targets = _jnp_targets(np.asarray(inputs["gt_boxes"], dtype=f32),
                           np.asarray(inputs["gt_cls"], dtype=np.int32))
    npos = [max(float(t[1].sum()), 1.0) for t in targets]

    proto = np.ascontiguousarray(inputs["proto_masks"], dtype=f32)
    gtm = np.ascontiguousarray(inputs["gt_masks"], dtype=f32)
    haz = np.ascontiguousarray(inputs["hazard_logits"], dtype=f32)
    ghz = np.asarray(inputs["gt_hazard"], dtype=np.int32)

    in_maps = []
    for core in range(NCORES):
        b0 = core * BL
        m = {
            "seg_x": proto[b0:b0+BL].reshape(BL, 128, SEG_F),
            "seg_t": gtm[b0:b0+BL].reshape(BL, 128, SEG_F),
            "objd3": objs[0][b0:b0+BL].reshape(128, 100),
            "objd4": objs[1][b0:b0+BL].reshape(128, 25),
            "objd5": objs[2][b0:b0+BL].reshape(100, 8),
            "haz_x": haz[b0:b0+BL],
            "haz_oh": np.eye(4, dtype=f32)[ghz[b0:b0+BL]],
            "haz_pr": PENALTY[ghz[b0:b0+BL]],
        }
        tbx_all = np.zeros((K, 12), f32)
        tcx_all = np.zeros((K, 3 * CN), f32)
        atm_all = np.zeros((K, 3 * CN), f32)
        bm_all = np.zeros((K, 3), f32)
        for i in range(3):
            HW = HWS[i]
            m[f"clst{i}"] = np.ascontiguousarray(
                clss[i][b0:b0+BL].transpose(0, 2, 3, 1).reshape(-1, CN))
            m[f"regt{i}"] = np.ascontiguousarray(
                regs[i][b0:b0+BL].transpose(0, 2, 3, 1).reshape(-1, 4))
            m[f"objf{i}"] = objs[i][b0:b0+BL].reshape(-1, 1)
            tgt_reg, tgt_obj, tgt_cls = targets[i]
            rows, tb, tcl = [], [], []
            for bl in range(BL):
                b = b0 + bl
                ys, xs = np.nonzero(tgt_obj[b])
                for gy, gx in zip(ys, xs):
                    rows.append(bl * HW + gy * DIMS[i][1] + gx)
                    tb.append(tgt_reg[b, gy, gx])
                    tcl.append(tgt_cls[b, gy, gx])
            n = len(rows)
            assert n <= K
            idx = np.zeros((K, 1), np.int32)
            if n:
                idx[:n, 0] = rows
                tbx = np.stack(tb)
                for ch in range(4):
                    tbx_all[:n, ch * 3 + i] = tbx[:, ch]
                tcx_all[:n, i*CN:(i+1)*CN] = np.stack(tcl)
                bm_all[:n, i] = 1.0
            m[f"idx{i}"] = idx
            atm_all[:, i*CN:(i+1)*CN] = (
                bm_all[:, i:i+1] * (0.75 - 0.5 * tcx_all[:, i*CN:(i+1)*CN]))
        m["tboxall"] = tbx_all
        m["tclsall"] = tcx_all
        m["atmall"] = atm_all
        m["bmaskall"] = bm_all
        in_maps.append(m)

    res = bass_utils.run_bass_kernel_spmd(nc, in_maps,
                                          core_ids=list(range(NCORES)))
    p = np.stack([res.results[c]["partials"] for c in range(NCORES)])
    p = p.astype(np.float64)

    # ---- host combine ----
    tb_ = to_ = tc_ = 0.0
    for i in range(3):
        H, W = DIMS[i]
        obj_sp = p[:, :, 34+i].sum()
        obj_pos = p[:, :, 37+i].sum()
        to_ += (obj_sp - obj_pos) / (B * H * W)
        tb_ += p[:, :, 40+i].sum() / npos[i]
        tc_ += p[:, :, 43+i].sum() / (npos[i] * NUM_CLASSES)
    tb_, to_, tc_ = tb_ / 3.0, to_ / 3.0, tc_ / 3.0

    sum_sp = p[:, :, 0:8].sum()
    sum_xt = p[:, :, 16:24].sum()
    bce = (sum_sp - sum_xt) / (B * SEG_ELEMS)
    dice = 0.0
    for sg in range(B):
        c, lo = sg // BL, sg % BL
        sum_r = p[c, :, 8+NCH*lo:8+NCH*(lo+1)].sum()
        P_b = SEG_ELEMS - sum_r
        T_b = p[c, 0, 32+lo]
        sum_rt = p[c, :, 24+NCH*lo:24+NCH*(lo+1)].sum()
        I_b = T_b - sum_rt
        dice += 1.0 - (2.0 * I_b + 1.0) / (P_b + T_b + 1.0)
    ts = (bce + dice / B) / 2.0

    th = p[:, 0:BL, 46].sum() / B
    thier = p[:, 0:BL, 47].sum() / B

    total = (L_BOX * tb_ + L_OBJ * to_ + L_CLS * tc_ + L_SEG * ts
             + L_HAZ * th + L_HIER * thier)
    return np.float32(total)
